# revision 1
# baseline (speedup 1.0000x reference)
"""Multi-head attention (N=2048, D=1024, H=16) on 8 TRN2 NeuronCores.

Sharding: tensor-parallel over heads (2 heads / core). x is replicated
(pre-transposed + pre-cast on host), each core computes QKV / scores /
softmax / PV / out-proj for its 2 heads, producing a partial (N, D)
projection output in fp16. The all-reduce over cores is the host-side
f64 sum of the 8 partials (+ b_proj), cast back to f32.

Device inputs (per core):
  xT      (D, N)    bf16 : x transposed (host prep)
  wqkvT   (D, 384)  bf16 : [Wq.T | Wk.T | Wv.T] column slices for 2 heads
  wpT     (128, D)  f32r : w_proj[:, core_cols].T
  bqkv    (128, 3)  f32  : [bq | bk | bv] slices
  out_part(N, D)    f16  : partial projection output

Per-core pipeline (streamed; emission order sets Tile priorities):
  QKV   bf16 matmuls, j-sliced and interleaved with block-0 attention so
        scores start while x is still streaming in -> Q.T/K.T (head-dim
        on partitions, bf16) and V.T
  V.T   --PE transpose--> V (seq on partitions) with ones columns
  scores S.T = K.T^T @ Q.T per head; both heads packed into the PE array
        via 64-row tiling (tile_position (0,0)/(64,0)), bf16, K=64 each
  exp   one ACT pass per m-chunk drains both heads' scores PSUM -> SBUF
        bf16 with the 1/sqrt(DH) scale folded in
  PV    [V|1]^T @ expS.T -> O'.T rows 0:64 + softmax rowsum in row 64
  divide DVE reciprocal + PE ones-matmul partition-broadcast + DVE muls
        (h1's mul writes partition-shifted)
  proj  O.T^T @ wpT (fp32r, full-rate) -> fp16 partial, one DMA per block;
        each block's projection is deferred into the next block's stream
"""

import os
import sys

import numpy as np

for _p in ("/opt/trn_rl_repo",):
    if os.path.isdir(_p) and _p not in sys.path:
        sys.path.insert(0, _p)

N, D, H = 2048, 1024, 16
DH = D // H                 # 64
NCORES = 8
HPC = H // NCORES           # 2 heads per core
P = 128
SCALE = 1.0 / DH ** 0.5

D_CHUNKS = D // P           # 8

# config knobs (overridable before run() for experiments)
USE_ROW_TILING = os.environ.get("ATTN_ROW_TILING", "1") == "1"
QKV_DTYPE = os.environ.get("ATTN_QKV_DTYPE", "bfloat16")   # float32r|float32|bfloat16
PROJ_DTYPE = os.environ.get("ATTN_PROJ_DTYPE", "float32r")
ES_BUFS = int(os.environ.get("ATTN_ES_BUFS", "6"))
SPS_BUFS = int(os.environ.get("ATTN_SPS_BUFS", "2"))
NB = int(os.environ.get("ATTN_NB", "512"))                 # query-block size
PV_FP8 = os.environ.get("ATTN_PV_FP8", "0") == "1"         # DoubleRow fp8 PV


def _build_nc(n=N, nb=NB):
    """Build the per-core Bass module (SPMD: identical program, per-core data)."""
    import concourse.bass as bass  # noqa: F401
    import concourse.mybir as mybir
    import concourse.tile as tile
    from concourse import bacc
    from concourse.masks import make_identity

    f32 = mybir.dt.float32
    bf16 = mybir.dt.bfloat16
    f32r = mybir.dt.float32r
    AF = mybir.ActivationFunctionType

    m_chunks = n // P
    n_blocks = n // nb

    dtmap = {"float32r": f32r, "float32": f32, "bfloat16": bf16}
    qkv_sb_dt = dtmap[QKV_DTYPE]
    proj_sb_dt = dtmap[PROJ_DTYPE]

    nc = bacc.Bacc(
        "TRN2",
        target_bir_lowering=False,
        debug=False,
        enable_asserts=True,
        num_devices=NCORES,
    )

    xT_d = nc.dram_tensor("xT", (D, n), qkv_sb_dt, kind="ExternalInput")
    wqkvT_d = nc.dram_tensor("wqkvT", (P, 3, D_CHUNKS, P), qkv_sb_dt, kind="ExternalInput")
    wpT_d = nc.dram_tensor("wpT", (P, D), proj_sb_dt, kind="ExternalInput")
    bqkv_d = nc.dram_tensor("bqkv", (P, 3), f32, kind="ExternalInput")
    f16 = mybir.dt.float16
    out_d = nc.dram_tensor("out_part", (n, D), f16, kind="ExternalOutput")

    with tile.TileContext(nc) as tc:
        with (
            tc.tile_pool(name="consts", bufs=1) as consts,
            tc.tile_pool(name="xpool", bufs=1) as xpool,
            tc.tile_pool(name="qkpool", bufs=1) as qkpool,
        ):
            # ---- inputs ----
            # wqkv loads per part (k first: it gates the first scores matmul);
            # x streams in (j, o) pieces so attention can start while x loads.
            wqkv_sb = consts.tile([P, 3, D_CHUNKS, P], qkv_sb_dt)
            wp_sb = consts.tile([P, D], proj_sb_dt)
            bqkv_sb = consts.tile([P, 3], f32)
            xT_sb = xpool.tile([P, D_CHUNKS, n], qkv_sb_dt)

            qw = min(512, n)
            n_j = n // qw
            nc.sync.dma_start(bqkv_sb[:], bqkv_d.ap())
            # part order: k(1), q(0), v(2); host sends wqkvT part-major so
            # each part's weight DMA is one contiguous 4KB run per partition
            PART_ORDER = (1, 0, 2)
            nc.sync.dma_start(wqkv_sb[:, 1], wqkvT_d.ap()[:, 1])

            def x_piece(j0, j1, o):
                nc.sync.dma_start(
                    xT_sb[:, o, j0 * qw:j1 * qw],
                    xT_d.ap()[o * P:(o + 1) * P, j0 * qw:j1 * qw],
                )

            for o in range(D_CHUNKS):
                x_piece(0, 1, o)
            nc.sync.dma_start(wqkv_sb[:, 0], wqkvT_d.ap()[:, 0])
            nc.sync.dma_start(wqkv_sb[:, 2], wqkvT_d.ap()[:, 2])
            for j in range(1, n_j):
                for o in range(D_CHUNKS):
                    x_piece(j, j + 1, o)
            nc.sync.dma_start(wp_sb[:], wpT_d.ap())

            ident = consts.tile([P, P], bf16)
            make_identity(nc, ident[:])
            # ones row at partition DH (aligned with PV rowsum row) for the
            # reciprocal partition-broadcast matmul (f32r for 1 cyc/row)
            ones_f32 = consts.tile([P, DH], f32)
            nc.gpsimd.memset(ones_f32[0:1, :], 1.0)
            ones_sb = consts.tile([P, DH], f32r)
            nc.vector.tensor_copy(ones_sb[0:1, :], ones_f32[0:1, :])

            # ---- persistent activations ----
            qT_sb = qkpool.tile([P, n], bf16)           # Q.T (head-dim on parts)
            kT_sb = qkpool.tile([P, n], bf16)           # K.T
            vT_sb = qkpool.tile([P, n], bf16)           # V.T (pre-transpose)
            fp8 = mybir.dt.float8e4
            if PV_FP8:
                # [pair, i, cols]: h0 at 0:65 (V|1), h1 at 80:145 (V|1);
                # row stride 160 and h offsets are 16B-aligned for DoubleRow
                v_sb = qkpool.tile([P, m_chunks // 2, 2, 160], fp8)
                nc.gpsimd.memset(v_sb[:, :, :, DH:DH + 1], 1.0)
                nc.gpsimd.memset(v_sb[:, :, :, 80 + DH:80 + DH + 1], 1.0)
            else:
                v_sb = qkpool.tile([P, m_chunks, 2 * (DH + 1)], bf16)  # [V_h0|1|V_h1|1]
                nc.gpsimd.memset(v_sb[:, :, DH:DH + 1], 1.0)
                nc.gpsimd.memset(v_sb[:, :, 2 * DH + 1:2 * DH + 2], 1.0)

            # ===== PSUM pools: one global budget, no phase aliasing =====
            # accp: QKV accumulators + transposes + proj + recip-bcast (1 bank)
            # sps:  scores tiles (2 banks each)
            # pvps: PV accumulators (1 bank each)
            # total: 2*1 + 2*2 + 2*1 = 8 banks
            accp = tc.alloc_tile_pool(name="accp", bufs=2, space="PSUM")
            sps = tc.alloc_tile_pool(name="sps", bufs=SPS_BUFS, space="PSUM")
            pvps = tc.alloc_tile_pool(name="pvps", bufs=2, space="PSUM")

            # ================= Phases: QKV + attention, interleaved ==========
            # Emission order drives Tile priorities:
            #   j0:(k,q,v)+transposes -> attn(b0, mc group j0) -> j1:(...) ...
            # then blocks 1..; each block's projection is deferred into the
            # next block's stream so it fills PE gaps instead of stalling ACT.
            dst = {0: qT_sb, 1: kT_sb, 2: vT_sb}

            def qkv_j(j, parts=PART_ORDER):
                for part in parts:
                    ps = accp.tile([P, qw], mybir.dt.float32, tag="acc",
                                   name=f"qkv_ps_{part}_{j}")
                    for o in range(D_CHUNKS):
                        nc.tensor.matmul(
                            ps[:],
                            wqkv_sb[:, part, o, :],
                            xT_sb[:, o, j * qw:(j + 1) * qw],
                            start=(o == 0),
                            stop=(o == D_CHUNKS - 1),
                        )
                    # drain with bias add (per-partition scalar), cast bf16
                    nc.vector.tensor_add(
                        dst[part][:, j * qw:(j + 1) * qw],
                        ps[:],
                        bqkv_sb[:, part:part + 1].broadcast_to([P, qw]),
                    )
                    if part == 2:
                        # V.T -> V for the m-chunks covered by this j slice
                        for mc in range(j * qw // P, (j + 1) * qw // P):
                            tp = accp.tile([P, P], bf16, tag="acc", name=f"tp_{mc}")
                            nc.tensor.transpose(
                                tp[:], vT_sb[:, mc * P:(mc + 1) * P], ident[:]
                            )
                            if PV_FP8:
                                g, i = mc // 2, mc % 2
                                nc.vector.tensor_copy(
                                    v_sb[:, g, i, 0:DH], tp[:, 0:DH]
                                )
                                nc.vector.tensor_copy(
                                    v_sb[:, g, i, 80:80 + DH], tp[:, DH:2 * DH]
                                )
                            else:
                                nc.vector.tensor_copy(v_sb[:, mc, 0:DH], tp[:, 0:DH])
                                nc.vector.tensor_copy(
                                    v_sb[:, mc, DH + 1:2 * DH + 1], tp[:, DH:2 * DH]
                                )

            # variable-size query blocks; small final block shrinks the tail
            bw = min(nb, n)
            blocks = [bw] * (n // bw)

            with (
                tc.tile_pool(name="espool", bufs=ES_BUFS) as espool,
                tc.tile_pool(name="opool", bufs=2) as opool,
                tc.tile_pool(name="outpool", bufs=2) as outpool,
                tc.tile_pool(name="rpool", bufs=2) as rpool,
            ):
                pps = accp

                def attn_sc(b, row0, nbb, mcs):
                    nsl = slice(row0, row0 + nbb)
                    out = []
                    es_pair = None
                    for mc in mcs:
                        s_ps = sps.tile([P, 2 * nbb], mybir.dt.float32, tag="s",
                                        name=f"s_ps_{b}_{mc}")
                        for h in range(HPC):
                            nc.tensor.matmul(
                                s_ps[:, h * nbb:(h + 1) * nbb],
                                kT_sb[h * DH:(h + 1) * DH, mc * P:(mc + 1) * P],
                                qT_sb[h * DH:(h + 1) * DH, nsl],
                                tile_position=(h * DH, 0) if USE_ROW_TILING else None,
                            )
                        if PV_FP8:
                            if mc % 2 == 0:
                                es_pair = espool.tile([P, 2, 2 * nbb], fp8, tag="es",
                                                      name=f"es_{b}_{mc}")
                            nc.scalar.activation(es_pair[:, mc % 2, :], s_ps[:],
                                                 AF.Exp, scale=SCALE)
                            if mc % 2 == 1:
                                out.append((mc // 2, es_pair))
                        else:
                            es = espool.tile([P, 2 * nbb], bf16, tag="es",
                                             name=f"es_{b}_{mc}")
                            nc.scalar.activation(es[:], s_ps[:], AF.Exp, scale=SCALE)
                            out.append((mc, es))
                    return out

                def attn_pv(nbb, pvs, mc_es):
                    if PV_FP8:
                        for g, es_pair in mc_es:
                            for h in range(HPC):
                                nc.tensor.matmul(
                                    pvs[h][0:DH + 1, :],
                                    v_sb[:, g, :, 80 * h:80 * h + DH + 1],
                                    es_pair[:, :, h * nbb:(h + 1) * nbb],
                                    start=(g == 0),
                                    stop=(g == m_chunks // 2 - 1),
                                    perf_mode=mybir.MatmulPerfMode.DoubleRow,
                                )
                        return
                    for mc, es in mc_es:
                        for h in range(HPC):
                            nc.tensor.matmul(
                                pvs[h][0:DH + 1, :],
                                v_sb[:, mc, h * (DH + 1):(h + 1) * (DH + 1)],
                                es[:, h * nbb:(h + 1) * nbb],
                                start=(mc == 0),
                                stop=(mc == m_chunks - 1),
                            )

                def attn_mc_group(b, row0, nbb, pvs, mcs):
                    attn_pv(nbb, pvs, attn_sc(b, row0, nbb, mcs))

                def division(b, nbb, pvs):
                    # O.T = O'.T / rowsum, heads stacked on partitions.
                    # h1's mul writes partition-shifted (verified on HW).
                    rt = rpool.tile([P, HPC * nbb], mybir.dt.float32r, tag="recip",
                                    name=f"rt_{b}")
                    rb = rpool.tile([P, HPC * nbb], mybir.dt.float32, tag="rbcast",
                                    name=f"rb_{b}")
                    oT = opool.tile([P, nbb], proj_sb_dt, tag="oT", name=f"oT_{b}")
                    for h in range(HPC):
                        hs = slice(h * nbb, (h + 1) * nbb)
                        with nc.allow_low_precision(reason="f32r recip, bcast mm"):
                            nc.vector.reciprocal(rt[0:1, hs], pvs[h][DH:DH + 1, :])
                        rb_ps = pps.tile([P, nbb], mybir.dt.float32, tag="acc",
                                         name=f"rb_ps_{b}_{h}")
                        nc.tensor.matmul(rb_ps[0:DH, :], ones_sb[0:1, :], rt[0:1, hs])
                        nc.vector.tensor_copy(rb[0:DH, hs], rb_ps[0:DH, :])
                        nc.vector.tensor_mul(
                            oT[h * DH:(h + 1) * DH, :],
                            pvs[h][0:DH, :],
                            rb[0:DH, hs],
                        )
                    return oT

                def projection(b, row0, nbb, oT, last=False):
                    nch = nbb // P
                    out_sb = outpool.tile([P, nch, D], f16, tag="out",
                                          name=f"out_{b}")
                    for j in range(nch):
                        for half in range(D // 512):
                            pp = pps.tile([P, 512], mybir.dt.float32, tag="acc",
                                          name=f"pp_{b}_{j}_{half}")
                            nc.tensor.matmul(
                                pp[:],
                                oT[:, j * P:(j + 1) * P],
                                wp_sb[:, half * 512:(half + 1) * 512],
                            )
                            dslc = out_sb[:, j, half * 512:(half + 1) * 512]
                            if last and (j % 2 == 1):
                                # ACT is idle in the tail; split drains across
                                # both engines to shorten the epilogue chain
                                nc.scalar.copy(dslc, pp[:])
                            else:
                                nc.vector.tensor_copy(dslc, pp[:])
                    nc.sync.dma_start(
                        out_d.ap()[row0:row0 + nbb, :].rearrange(
                            "(c p) d -> p c d", p=P
                        ),
                        out_sb[:],
                    )

                pending = None   # (b, row0, nbb, oT) awaiting projection
                row0 = 0
                for b, nbb in enumerate(blocks):
                    pvs = [
                        pvps.tile([P, nbb], mybir.dt.float32, tag="pv",
                                  name=f"pv_{b}_{h}")
                        for h in range(HPC)
                    ]
                    if b == 0:
                        # fine interleave with QKV j-sweeps: k-slice -> scores
                        # -> v-slice(+transpose) -> PV; q for j0 plus block 1's
                        # q-slice at the end (each later block's q-slice is
                        # pre-emitted inside the previous block's stream so it
                        # never sits on the block-boundary critical path)
                        mcs_per_j = qw // P
                        for j in range(n_j):
                            qkv_j(j, parts=(1, 0) if j == 0 else (1,))
                            mc_es = attn_sc(b, row0, nbb,
                                            range(j * mcs_per_j, (j + 1) * mcs_per_j))
                            qkv_j(j, parts=(2,))
                            attn_pv(nbb, pvs, mc_es)
                        if n_j > 1:
                            qkv_j(1, parts=(0,))
                    else:
                        # deferred projection + next block's q-slice interleave
                        # after the first mcs (PE slack while ACT streams exps)
                        split = min(4, m_chunks)
                        attn_mc_group(b, row0, nbb, pvs, range(0, split))
                        if pending is not None:
                            projection(*pending)
                            pending = None
                        if b + 1 < n_j:
                            qkv_j(b + 1, parts=(0,))
                        attn_mc_group(b, row0, nbb, pvs, range(split, m_chunks))
                    oT = division(b, nbb, pvs)
                    if pending is not None:
                        projection(*pending)
                        pending = None
                    pending = (b, row0, nbb, oT)
                    row0 += nbb
                projection(*pending, last=True)

            pvps.release()
            sps.release()
            accp.release()

    nc.compile()
    return nc


def _host_prep(x, w_qkv, b_qkv, w_proj, n=N):
    """Per-core input maps (dtypes match the DRAM tensor declarations)."""
    import ml_dtypes

    qkv_np = (ml_dtypes.bfloat16 if QKV_DTYPE == "bfloat16" else np.float32)
    proj_np = (ml_dtypes.bfloat16 if PROJ_DTYPE == "bfloat16" else np.float32)
    xT = np.ascontiguousarray(x.T.astype(qkv_np))
    in_maps = []
    for c in range(NCORES):
        wq = w_qkv[0 * D + c * P:0 * D + (c + 1) * P, :]
        wk = w_qkv[1 * D + c * P:1 * D + (c + 1) * P, :]
        wv = w_qkv[2 * D + c * P:2 * D + (c + 1) * P, :]
        # part-major [p, part, o, c]: contiguous per-part weight DMAs
        wqkvT = np.ascontiguousarray(
            np.stack(
                [a.T.reshape(D_CHUNKS, P, P).transpose(1, 0, 2) for a in (wq, wk, wv)],
                axis=1,
            ).astype(qkv_np)
        )
        wpT = np.ascontiguousarray(w_proj[:, c * P:(c + 1) * P].T.astype(proj_np))
        bq = b_qkv[0 * D + c * P:0 * D + (c + 1) * P]
        bk = b_qkv[1 * D + c * P:1 * D + (c + 1) * P]
        bv = b_qkv[2 * D + c * P:2 * D + (c + 1) * P]
        bqkv = np.ascontiguousarray(
            np.stack([bq, bk, bv], axis=1).astype(np.float32)
        )
        in_maps.append({"xT": xT, "wqkvT": wqkvT, "wpT": wpT, "bqkv": bqkv})
    return in_maps


_NC_CACHE = {}


def run(x, w_qkv, b_qkv, w_proj, b_proj, trace=False, n=N, nb=None, **spmd_kwargs):
    from concourse.bass_utils import run_bass_kernel_spmd

    if nb is None:
        nb = NB
    key = (n, nb, USE_ROW_TILING, QKV_DTYPE, PROJ_DTYPE, ES_BUFS, SPS_BUFS, PV_FP8)
    if key not in _NC_CACHE:
        _NC_CACHE[key] = _build_nc(n=n, nb=nb)
    nc = _NC_CACHE[key]

    in_maps = _host_prep(
        np.asarray(x), np.asarray(w_qkv), np.asarray(b_qkv), np.asarray(w_proj), n=n
    )
    results = run_bass_kernel_spmd(
        nc, in_maps, core_ids=list(range(NCORES)), trace=trace, **spmd_kwargs
    )
    acc = np.zeros((n, D), dtype=np.float64)
    for c in range(NCORES):
        acc += results.results[c]["out_part"].astype(np.float64)
    acc += np.asarray(b_proj).astype(np.float64)
    return acc.astype(np.float32), results


def kernel(x, w_qkv, b_qkv, w_proj, b_proj):
    out, _ = run(x, w_qkv, b_qkv, w_proj, b_proj, trace=False)
    return out

